# revision 3
# baseline (speedup 1.0000x reference)
"""Trainium2 Bass kernel for CrossAttention (silu-scored, masked) sharded over
8 NeuronCores.

Problem (full shapes):
    query/key/value: [2, 2048, 1024] f32, mask: [2, 1, 2048, 2048] int32
    Wq/Wk/Wv/Wo: [1024, 1024] f32, bq/bk/bv: [1024] f32
    out = silu(mask((q @ k.T) * scale)) @ v heads-merged @ Wo.T

Sharding: core c handles batch b = c // 4 and heads 4*(c%4) .. 4*(c%4)+3
(data parallel on B, tensor parallel on heads).  Each core computes a
row-parallel partial of the O-projection; the host sums the 4 partials per
batch.  No cross-device communication.

Per-core device program (all computed transposed so the PE contracts on the
partition dim):
  qT = SCALE*(Wq_loc @ x_q)   [256, 2048] f32   (activations pre-transposed on host)
  kT =        Wk_loc @ x_k    [256, 2048] f32
  vT =        Wv_loc @ x_v -> PE-transpose -> v [2048, 256] bf16
  per head h:  sT[sk, sq] = kT_h.T-contract  (K=64)      -> PSUM
               attn = silu(sT) * maskT      (bf16)
               ctxT_h[d, sq] += v_h[sk,:].T @ attn       (accumulate over sk)
  partial = ctxT.T @ Wo_locT                [2048, 1024] f32
"""

import os
import numpy as np
import ml_dtypes

B = 2
S = 2048
HID = 1024
HEADS = 16
DH = 64
N_CORES = 8
GROUPS = 4          # head-groups (cores per batch)
NH_LOC = HEADS // GROUPS  # 4 heads per core
DLOC = NH_LOC * DH        # 256 local features
SCALE = DH ** -0.5

F32 = np.float32
BF16 = ml_dtypes.bfloat16

_COMPILED = {}


def build_program():
    import concourse.bass as bass
    import concourse.tile as tile
    from concourse import bacc, mybir
    from concourse.masks import make_identity

    f32 = mybir.dt.float32
    bf16 = mybir.dt.bfloat16

    nc = bacc.Bacc("TRN2", target_bir_lowering=False, debug=False,
                   enable_asserts=False, num_devices=N_CORES)

    xq = nc.dram_tensor("xq", [HID, S], f32, kind="ExternalInput").ap()
    xk = nc.dram_tensor("xk", [HID, S], f32, kind="ExternalInput").ap()
    xv = nc.dram_tensor("xv", [HID, S], f32, kind="ExternalInput").ap()
    mk = nc.dram_tensor("mk", [S, S], bf16, kind="ExternalInput").ap()
    wq = nc.dram_tensor("wq", [HID, DLOC], f32, kind="ExternalInput").ap()
    wk = nc.dram_tensor("wk", [HID, DLOC], f32, kind="ExternalInput").ap()
    wv = nc.dram_tensor("wv", [HID, DLOC], f32, kind="ExternalInput").ap()
    wo = nc.dram_tensor("wo", [DLOC, HID], f32, kind="ExternalInput").ap()
    bq = nc.dram_tensor("bq", [DLOC, 1], f32, kind="ExternalInput").ap()
    bk = nc.dram_tensor("bk", [DLOC, 1], f32, kind="ExternalInput").ap()
    bv = nc.dram_tensor("bv", [DLOC, 1], f32, kind="ExternalInput").ap()
    out = nc.dram_tensor("out", [S, HID], f32, kind="ExternalOutput").ap()

    SILU = mybir.ActivationFunctionType.Silu
    MUL = mybir.AluOpType.mult
    ADD = mybir.AluOpType.add

    with tile.TileContext(nc) as tc:
        with (
            tc.tile_pool(name="res", bufs=1) as res,
            tc.tile_pool(name="io", bufs=4) as io,
        ):
            # ---- resident SBUF tensors ----
            mask_sb = res.tile([128, 16 * S], bf16, tag="mask", name="mask_sb")   # [sk-tile j][p, j*2048+sq]
            qt = [res.tile([128, S], f32, tag=f"qt{m}", name=f"qt{m}") for m in range(2)]
            kt = [res.tile([128, S], f32, tag=f"kt{m}", name=f"kt{m}") for m in range(2)]
            vt_bf = [res.tile([128, S], bf16, tag=f"vt{m}", name=f"vt{m}") for m in range(2)]
            v_bf = res.tile([128, 16 * DLOC], bf16, tag="vbf", name="v_bf")    # v[j*128+p, d] at [p, j*256+d]
            ctxt = [res.tile([128, S], f32, tag=f"ctxt{m}", name=f"ctxt{m}") for m in range(2)]
            wo_sb = [res.tile([128, HID], f32, tag=f"wo{k}", name=f"wo_sb{k}") for k in range(2)]
            ident = res.tile([128, 128], bf16, tag="ident", name="ident")
            b_sb = {}
            for nm, src in (("bq", bq), ("bk", bk), ("bv", bv)):
                b_sb[nm] = [res.tile([128, 1], f32, tag=f"{nm}{m}", name=f"{nm}_sb{m}") for m in range(2)]
                for m in range(2):
                    nc.sync.dma_start(out=b_sb[nm][m][:, :], in_=src[m * 128:(m + 1) * 128, :])

            nc.sync.dma_start(
                out=mask_sb[:, :].rearrange("p (j q) -> p j q", j=16),
                in_=mk.rearrange("(j p) q -> p j q", p=128),
            )
            for k in range(2):
                nc.sync.dma_start(out=wo_sb[k][:, :], in_=wo[k * 128:(k + 1) * 128, :])
            make_identity(nc, ident[:, :])

            # ---- Phase A: QKV projections ----
            with (
                tc.tile_pool(name="psA", bufs=2, space="PSUM") as psA,
                tc.tile_pool(name="wA", bufs=2) as wA,
            ):
                projs = [
                    ("q", xq, wq, "bq", SCALE, qt),
                    ("k", xk, wk, "bk", 1.0, kt),
                    ("v", xv, wv, "bv", 1.0, vt_bf),
                ]
                for nm, x_ap, w_ap, bnm, scl, dst in projs:
                    w_sb = wA.tile([128, 8 * DLOC], f32, tag="w", name=f"w_{nm}")
                    nc.sync.dma_start(
                        out=w_sb[:, :].rearrange("p (k m) -> p k m", k=8),
                        in_=w_ap.rearrange("(k p) m -> p k m", p=128),
                    )
                    for n in range(4):
                        pacc = [psA.tile([128, 512], f32, tag=f"pacc{m}", name=f"pacc{m}") for m in range(2)]
                        for k in range(8):
                            rhs = io.tile([128, 512], f32, tag="xrhs", name="xrhs")
                            nc.sync.dma_start(
                                out=rhs[:, :],
                                in_=x_ap[k * 128:(k + 1) * 128, n * 512:(n + 1) * 512],
                            )
                            for m in range(2):
                                nc.tensor.matmul(
                                    pacc[m][:, :],
                                    lhsT=w_sb[:, k * DLOC + m * 128: k * DLOC + (m + 1) * 128],
                                    rhs=rhs[:, :],
                                    start=(k == 0), stop=(k == 7),
                                )
                        for m in range(2):
                            nc.vector.tensor_scalar(
                                out=dst[m][:, n * 512:(n + 1) * 512],
                                in0=pacc[m][:, :],
                                scalar1=float(scl),
                                scalar2=b_sb[bnm][m][:, 0:1],
                                op0=MUL, op1=ADD,
                            )
                # transpose vT -> v (natural), bf16
                for m in range(2):
                    for j in range(16):
                        tr = psA.tile([128, 128], bf16, tag="tr", name="tr")
                        nc.tensor.transpose(
                            tr[:, :], vt_bf[m][:, j * 128:(j + 1) * 128], ident[:, :]
                        )
                        nc.vector.tensor_copy(
                            out=v_bf[:, j * DLOC + m * 128: j * DLOC + (m + 1) * 128],
                            in_=tr[:, :],
                        )

            # ---- Phase B: attention per head ----
            with (
                tc.tile_pool(name="psT", bufs=2, space="PSUM") as psT,
                tc.tile_pool(name="psAcc", bufs=1, space="PSUM") as psAcc,
                tc.tile_pool(name="attp", bufs=3) as attp,
            ):
                for h in range(NH_LOC):
                    t_i = h // 2
                    po = (h % 2) * 64
                    acc = [psAcc.tile([64, 512], f32, tag=f"acc{s}", name=f"acc{s}") for s in range(4)]
                    for j in range(16):
                        for half in range(2):
                            sT = psT.tile([128, 1024], f32, tag="sT", name="sT")
                            for s2 in range(2):
                                sqb = half * 2 + s2
                                nc.tensor.matmul(
                                    sT[:, s2 * 512:(s2 + 1) * 512],
                                    lhsT=kt[t_i][po:po + 64, j * 128:(j + 1) * 128],
                                    rhs=qt[t_i][po:po + 64, sqb * 512:(sqb + 1) * 512],
                                    start=True, stop=True,
                                )
                            att = attp.tile([128, 1024], bf16, tag="att", name="att")
                            nc.scalar.activation(att[:, :], sT[:, :], SILU)
                            attn = attp.tile([128, 1024], bf16, tag="attn", name="attn")
                            nc.vector.tensor_mul(
                                out=attn[:, :],
                                in0=att[:, :],
                                in1=mask_sb[:, j * S + half * 1024: j * S + half * 1024 + 1024],
                            )
                            for s2 in range(2):
                                sqb = half * 2 + s2
                                nc.tensor.matmul(
                                    acc[sqb][:, :],
                                    lhsT=v_bf[:, j * DLOC + h * 64: j * DLOC + h * 64 + 64],
                                    rhs=attn[:, s2 * 512:(s2 + 1) * 512],
                                    start=(j == 0), stop=(j == 15),
                                )
                    for sqb in range(4):
                        nc.vector.tensor_copy(
                            out=ctxt[t_i][po:po + 64, sqb * 512:(sqb + 1) * 512],
                            in_=acc[sqb][:, :],
                        )

            # ---- Phase C: O projection ----
            with (
                tc.tile_pool(name="psC", bufs=4, space="PSUM") as psC,
                tc.tile_pool(name="oev", bufs=4) as oev,
            ):
                for mb in range(16):
                    for n2 in range(2):
                        pot = psC.tile([128, 512], f32, tag="po", name="pot")
                        for k in range(2):
                            nc.tensor.matmul(
                                pot[:, :],
                                lhsT=ctxt[k][:, mb * 128:(mb + 1) * 128],
                                rhs=wo_sb[k][:, n2 * 512:(n2 + 1) * 512],
                                start=(k == 0), stop=(k == 1),
                            )
                        ev = oev.tile([128, 512], f32, tag="oev", name="ev")
                        if mb % 2 == 0:
                            nc.vector.tensor_copy(out=ev[:, :], in_=pot[:, :])
                        else:
                            nc.scalar.copy(out=ev[:, :], in_=pot[:, :])
                        nc.sync.dma_start(
                            out=out[mb * 128:(mb + 1) * 128, n2 * 512:(n2 + 1) * 512],
                            in_=ev[:, :],
                        )

    nc.compile()
    return nc


def get_program():
    if "nc" not in _COMPILED:
        _COMPILED["nc"] = build_program()
    return _COMPILED["nc"]


def make_in_maps(query, key, value, mask, Wq, bq, Wk, bk, Wv, bv, Wo):
    """Host-side sharding/layout prep: one input map per core."""
    query = np.asarray(query, dtype=F32)
    key = np.asarray(key, dtype=F32)
    value = np.asarray(value, dtype=F32)
    mask = np.asarray(mask)
    in_maps = []
    maskT = [np.ascontiguousarray(mask[b, 0].T).astype(BF16) for b in range(B)]
    xqT = [np.ascontiguousarray(query[b].T) for b in range(B)]
    xkT = [np.ascontiguousarray(key[b].T) for b in range(B)]
    xvT = [np.ascontiguousarray(value[b].T) for b in range(B)]
    for c in range(N_CORES):
        b = c // GROUPS
        g = c % GROUPS
        rs = slice(g * DLOC, (g + 1) * DLOC)
        in_maps.append({
            "xq": xqT[b],
            "xk": xkT[b],
            "xv": xvT[b],
            "mk": maskT[b],
            "wq": np.ascontiguousarray(np.asarray(Wq, F32)[rs, :].T),
            "wk": np.ascontiguousarray(np.asarray(Wk, F32)[rs, :].T),
            "wv": np.ascontiguousarray(np.asarray(Wv, F32)[rs, :].T),
            "wo": np.ascontiguousarray(np.asarray(Wo, F32)[:, rs].T),
            "bq": (SCALE * np.asarray(bq, F32)[rs]).reshape(DLOC, 1),
            "bk": np.asarray(bk, F32)[rs].reshape(DLOC, 1),
            "bv": np.asarray(bv, F32)[rs].reshape(DLOC, 1),
        })
    return in_maps


def run_on_device(in_maps, trace=False, tmpdir=None):
    from concourse.bass_utils import run_bass_kernel_spmd
    nc = get_program()
    kwargs = {}
    if trace:
        kwargs.update(trace=True, tmpdir=tmpdir)
    return run_bass_kernel_spmd(nc, in_maps, list(range(N_CORES)), **kwargs)


def assemble_output(results):
    out = np.zeros((B, S, HID), dtype=F32)
    for c in range(N_CORES):
        out[c // GROUPS] += results[c]["out"]
    return out


def kernel(query, key, value, mask, Wq, bq, Wk, bk, Wv, bv, Wo):
    in_maps = make_in_maps(query, key, value, mask, Wq, bq, Wk, bk, Wv, bv, Wo)
    res = run_on_device(in_maps)
    return assemble_output(res.results)


# revision 4
# speedup vs baseline: 2.5677x; 2.5677x over previous
"""Trainium2 Bass kernel for CrossAttention (silu-scored, masked) sharded over
8 NeuronCores.

Problem (full shapes):
    query/key/value: [2, 2048, 1024] f32, mask: [2, 1, 2048, 2048] int32
    Wq/Wk/Wv/Wo: [1024, 1024] f32, bq/bk/bv: [1024] f32
    out = silu(mask((q @ k.T) * scale)) @ v heads-merged @ Wo.T

Sharding: core c handles batch b = c // 4 and heads 4*(c%4) .. 4*(c%4)+3
(data parallel on B, tensor parallel on heads).  Each core computes a
row-parallel partial of the O-projection; the host sums the 4 partials per
batch.  No cross-device communication.

Per-core device program (all computed transposed so the PE contracts on the
partition dim):
  qT = SCALE*(Wq_loc @ x_q)   [256, 2048] f32   (activations pre-transposed on host)
  kT =        Wk_loc @ x_k    [256, 2048] f32
  vT =        Wv_loc @ x_v -> PE-transpose -> v [2048, 256] bf16
  per head h:  sT[sk, sq] = kT_h.T-contract  (K=64)      -> PSUM
               attn = silu(sT) * maskT      (bf16)
               ctxT_h[d, sq] += v_h[sk,:].T @ attn       (accumulate over sk)
  partial = ctxT.T @ Wo_locT                [2048, 1024] f32
"""

import os
import numpy as np
import ml_dtypes

B = 2
S = 2048
HID = 1024
HEADS = 16
DH = 64
N_CORES = 8
GROUPS = 4          # head-groups (cores per batch)
NH_LOC = HEADS // GROUPS  # 4 heads per core
DLOC = NH_LOC * DH        # 256 local features
SCALE = DH ** -0.5

F32 = np.float32
BF16 = ml_dtypes.bfloat16

_COMPILED = {}


def build_program():
    import concourse.bass as bass
    import concourse.tile as tile
    from concourse import bacc, mybir
    from concourse.masks import make_identity

    f32 = mybir.dt.float32
    bf16 = mybir.dt.bfloat16

    nc = bacc.Bacc("TRN2", target_bir_lowering=False, debug=False,
                   enable_asserts=False, num_devices=N_CORES)

    xq = nc.dram_tensor("xq", [HID, S], bf16, kind="ExternalInput").ap()
    xk = nc.dram_tensor("xk", [HID, S], bf16, kind="ExternalInput").ap()
    xv = nc.dram_tensor("xv", [HID, S], bf16, kind="ExternalInput").ap()
    mk = nc.dram_tensor("mk", [S, S], bf16, kind="ExternalInput").ap()
    wq = nc.dram_tensor("wq", [HID, DLOC], bf16, kind="ExternalInput").ap()
    wk = nc.dram_tensor("wk", [HID, DLOC], bf16, kind="ExternalInput").ap()
    wv = nc.dram_tensor("wv", [HID, DLOC], bf16, kind="ExternalInput").ap()
    wo = nc.dram_tensor("wo", [DLOC, HID], bf16, kind="ExternalInput").ap()
    bq = nc.dram_tensor("bq", [DLOC, 1], f32, kind="ExternalInput").ap()
    bk = nc.dram_tensor("bk", [DLOC, 1], f32, kind="ExternalInput").ap()
    bv = nc.dram_tensor("bv", [DLOC, 1], f32, kind="ExternalInput").ap()
    out = nc.dram_tensor("out", [S, HID], f32, kind="ExternalOutput").ap()

    SILU = mybir.ActivationFunctionType.Silu
    MUL = mybir.AluOpType.mult
    ADD = mybir.AluOpType.add

    with tile.TileContext(nc) as tc:
        with (
            tc.tile_pool(name="res", bufs=1) as res,
            tc.tile_pool(name="io", bufs=4) as io,
        ):
            # ---- resident SBUF tensors ----
            mask_sb = res.tile([128, 16 * S], bf16, tag="mask", name="mask_sb")   # [sk-tile j][p, j*2048+sq]
            qt = [res.tile([128, S], bf16, tag=f"qt{m}", name=f"qt{m}") for m in range(2)]
            kt = [res.tile([128, S], bf16, tag=f"kt{m}", name=f"kt{m}") for m in range(2)]
            vt_bf = [res.tile([128, S], bf16, tag=f"vt{m}", name=f"vt{m}") for m in range(2)]
            v_bf = res.tile([128, 16 * DLOC], bf16, tag="vbf", name="v_bf")    # v[j*128+p, d] at [p, j*256+d]
            ctxt = [res.tile([128, S], bf16, tag=f"ctxt{m}", name=f"ctxt{m}") for m in range(2)]
            wo_sb = [res.tile([128, HID], bf16, tag=f"wo{k}", name=f"wo_sb{k}") for k in range(2)]
            ident = res.tile([128, 128], bf16, tag="ident", name="ident")
            b_sb = {}
            for nm, src in (("bq", bq), ("bk", bk), ("bv", bv)):
                b_sb[nm] = [res.tile([128, 1], f32, tag=f"{nm}{m}", name=f"{nm}_sb{m}") for m in range(2)]
                for m in range(2):
                    nc.sync.dma_start(out=b_sb[nm][m][:, :], in_=src[m * 128:(m + 1) * 128, :])

            nc.sync.dma_start(
                out=mask_sb[:, :].rearrange("p (j q) -> p j q", j=16),
                in_=mk.rearrange("(j p) q -> p j q", p=128),
            )
            for k in range(2):
                nc.sync.dma_start(out=wo_sb[k][:, :], in_=wo[k * 128:(k + 1) * 128, :])
            make_identity(nc, ident[:, :])

            # ---- Phase A: QKV projections ----
            with (
                tc.tile_pool(name="psA", bufs=2, space="PSUM") as psA,
                tc.tile_pool(name="wA", bufs=2) as wA,
            ):
                projs = [
                    ("q", xq, wq, "bq", SCALE, qt),
                    ("k", xk, wk, "bk", 1.0, kt),
                    ("v", xv, wv, "bv", 1.0, vt_bf),
                ]
                for nm, x_ap, w_ap, bnm, scl, dst in projs:
                    w_sb = wA.tile([128, 8 * DLOC], bf16, tag="w", name=f"w_{nm}")
                    nc.sync.dma_start(
                        out=w_sb[:, :].rearrange("p (k m) -> p k m", k=8),
                        in_=w_ap.rearrange("(k p) m -> p k m", p=128),
                    )
                    for n in range(4):
                        pacc = [psA.tile([128, 512], f32, tag=f"pacc{m}", name=f"pacc{m}") for m in range(2)]
                        for k in range(8):
                            rhs = io.tile([128, 512], bf16, tag="xrhs", name="xrhs")
                            nc.sync.dma_start(
                                out=rhs[:, :],
                                in_=x_ap[k * 128:(k + 1) * 128, n * 512:(n + 1) * 512],
                            )
                            for m in range(2):
                                nc.tensor.matmul(
                                    pacc[m][:, :],
                                    lhsT=w_sb[:, k * DLOC + m * 128: k * DLOC + (m + 1) * 128],
                                    rhs=rhs[:, :],
                                    start=(k == 0), stop=(k == 7),
                                )
                        for m in range(2):
                            nc.vector.tensor_scalar(
                                out=dst[m][:, n * 512:(n + 1) * 512],
                                in0=pacc[m][:, :],
                                scalar1=float(scl),
                                scalar2=b_sb[bnm][m][:, 0:1],
                                op0=MUL, op1=ADD,
                            )
                # transpose vT -> v (natural), bf16
                for m in range(2):
                    for j in range(16):
                        tr = psA.tile([128, 128], bf16, tag="tr", name="tr")
                        nc.tensor.transpose(
                            tr[:, :], vt_bf[m][:, j * 128:(j + 1) * 128], ident[:, :]
                        )
                        nc.vector.tensor_copy(
                            out=v_bf[:, j * DLOC + m * 128: j * DLOC + (m + 1) * 128],
                            in_=tr[:, :],
                        )

            # ---- Phase B: attention per head ----
            with (
                tc.tile_pool(name="psT", bufs=2, space="PSUM") as psT,
                tc.tile_pool(name="psAcc", bufs=1, space="PSUM") as psAcc,
                tc.tile_pool(name="attp", bufs=3) as attp,
            ):
                for h in range(NH_LOC):
                    t_i = h // 2
                    po = (h % 2) * 64
                    acc = [psAcc.tile([64, 512], f32, tag=f"acc{s}", name=f"acc{s}") for s in range(4)]
                    for j in range(16):
                        for half in range(2):
                            sT = psT.tile([128, 1024], f32, tag="sT", name="sT")
                            for s2 in range(2):
                                sqb = half * 2 + s2
                                nc.tensor.matmul(
                                    sT[:, s2 * 512:(s2 + 1) * 512],
                                    lhsT=kt[t_i][po:po + 64, j * 128:(j + 1) * 128],
                                    rhs=qt[t_i][po:po + 64, sqb * 512:(sqb + 1) * 512],
                                    start=True, stop=True,
                                )
                            att = attp.tile([128, 1024], bf16, tag="att", name="att")
                            nc.scalar.activation(att[:, :], sT[:, :], SILU)
                            attn = attp.tile([128, 1024], bf16, tag="attn", name="attn")
                            nc.vector.tensor_mul(
                                out=attn[:, :],
                                in0=att[:, :],
                                in1=mask_sb[:, j * S + half * 1024: j * S + half * 1024 + 1024],
                            )
                            for s2 in range(2):
                                sqb = half * 2 + s2
                                nc.tensor.matmul(
                                    acc[sqb][:, :],
                                    lhsT=v_bf[:, j * DLOC + h * 64: j * DLOC + h * 64 + 64],
                                    rhs=attn[:, s2 * 512:(s2 + 1) * 512],
                                    start=(j == 0), stop=(j == 15),
                                )
                    for sqb in range(4):
                        nc.vector.tensor_copy(
                            out=ctxt[t_i][po:po + 64, sqb * 512:(sqb + 1) * 512],
                            in_=acc[sqb][:, :],
                        )

            # ---- Phase C: O projection ----
            with (
                tc.tile_pool(name="psC", bufs=4, space="PSUM") as psC,
                tc.tile_pool(name="oev", bufs=4) as oev,
            ):
                for mb in range(16):
                    for n2 in range(2):
                        pot = psC.tile([128, 512], f32, tag="po", name="pot")
                        for k in range(2):
                            nc.tensor.matmul(
                                pot[:, :],
                                lhsT=ctxt[k][:, mb * 128:(mb + 1) * 128],
                                rhs=wo_sb[k][:, n2 * 512:(n2 + 1) * 512],
                                start=(k == 0), stop=(k == 1),
                            )
                        ev = oev.tile([128, 512], f32, tag="oev", name="ev")
                        if mb % 2 == 0:
                            nc.vector.tensor_copy(out=ev[:, :], in_=pot[:, :])
                        else:
                            nc.scalar.copy(out=ev[:, :], in_=pot[:, :])
                        nc.sync.dma_start(
                            out=out[mb * 128:(mb + 1) * 128, n2 * 512:(n2 + 1) * 512],
                            in_=ev[:, :],
                        )

    nc.compile()
    return nc


def get_program():
    if "nc" not in _COMPILED:
        _COMPILED["nc"] = build_program()
    return _COMPILED["nc"]


def make_in_maps(query, key, value, mask, Wq, bq, Wk, bk, Wv, bv, Wo):
    """Host-side sharding/layout prep: one input map per core."""
    query = np.asarray(query, dtype=F32)
    key = np.asarray(key, dtype=F32)
    value = np.asarray(value, dtype=F32)
    mask = np.asarray(mask)
    in_maps = []
    maskT = [np.ascontiguousarray(mask[b, 0].T).astype(BF16) for b in range(B)]
    xqT = [np.ascontiguousarray(query[b].T).astype(BF16) for b in range(B)]
    xkT = [np.ascontiguousarray(key[b].T).astype(BF16) for b in range(B)]
    xvT = [np.ascontiguousarray(value[b].T).astype(BF16) for b in range(B)]
    for c in range(N_CORES):
        b = c // GROUPS
        g = c % GROUPS
        rs = slice(g * DLOC, (g + 1) * DLOC)
        in_maps.append({
            "xq": xqT[b],
            "xk": xkT[b],
            "xv": xvT[b],
            "mk": maskT[b],
            "wq": np.ascontiguousarray(np.asarray(Wq, F32)[rs, :].T).astype(BF16),
            "wk": np.ascontiguousarray(np.asarray(Wk, F32)[rs, :].T).astype(BF16),
            "wv": np.ascontiguousarray(np.asarray(Wv, F32)[rs, :].T).astype(BF16),
            "wo": np.ascontiguousarray(np.asarray(Wo, F32)[:, rs].T).astype(BF16),
            "bq": (SCALE * np.asarray(bq, F32)[rs]).reshape(DLOC, 1),
            "bk": np.asarray(bk, F32)[rs].reshape(DLOC, 1),
            "bv": np.asarray(bv, F32)[rs].reshape(DLOC, 1),
        })
    return in_maps


def run_on_device(in_maps, trace=False, tmpdir=None):
    from concourse.bass_utils import run_bass_kernel_spmd
    nc = get_program()
    kwargs = {}
    if trace:
        kwargs.update(trace=True, tmpdir=tmpdir)
    return run_bass_kernel_spmd(nc, in_maps, list(range(N_CORES)), **kwargs)


def assemble_output(results):
    out = np.zeros((B, S, HID), dtype=F32)
    for c in range(N_CORES):
        out[c // GROUPS] += results[c]["out"]
    return out


def kernel(query, key, value, mask, Wq, bq, Wk, bk, Wv, bv, Wo):
    in_maps = make_in_maps(query, key, value, mask, Wq, bq, Wk, bk, Wv, bv, Wo)
    res = run_on_device(in_maps)
    return assemble_output(res.results)
